# revision 4
# baseline (speedup 1.0000x reference)
"""BuildCostVolume kernel for 8 Trainium2 NeuronCores (v2).

Decomposition: the 9 strided dilated convs (disparities d=-4..4) reduce to
729 taps (d,u,v): cost[b,co,d,h,w] = sum_{ci,u,v} Wd[d][co,ci,u,v] *
X[b,ci,u,v, h+d(4-u), w+d(4-v)] where X is the view-decomposed light field
(X[b,ci,u,v,h,w] = x[b,ci,9h+u,9w+v], zero outside) and Wd flips (u,v) for
d>0.  Each tap is a K=64(ci) x M=64(co) matmul over spatial positions.

Sharding: core = (batch b, h-half).  All 81 views' h-windows live in SBUF as
41 view-pair tiles ([128, R*48] bf16, two views on partition halves).

v2 structure (vs v1):
- Weights: only 2 resident planes (d<=0 unflipped, d>0 flipped) instead of 9
  per-d copies; loaded once (saves ~4.7 MB DMA and all per-d weight waits).
- Slots: 27 (d, 8-row sub) outputs.  Diagonal position pairing: a slot's
  psum tile takes rh0-taps via PE column group 0 into partitions 0:64 and
  rh1-taps via column group 1 into 64:128 -> only 2 partial regions per
  slot (1 psum bank), 2-op DVE evac, 6+ slots live concurrently.
- Phases: slot triples walk the taps once per tile-subset; one LDWEIGHTS
  per (tap, position) serves 3 matmuls (k=3 measured fastest: 49.5 ns/MM
  vs 65 at k=1).  Psum banks rotate through 8 tags across phases.
- DMA overlap: tiles split into NSPLIT subsets, each led by a full-width
  v=4 seed pair (full-box first matmul per psum region sets has_written).
  Phase (triple x subset) accumulates that subset's taps; partial sums
  flush to bf16 SBUF accumulators so early phases run while later tiles
  stream in.
- h-clip: matmuls touch only output rows whose shifted input row is inside
  the 48-row view (elsewhere x is zero-padded); fully-clipped taps skip.
"""

import numpy as np
import ml_dtypes

A = 9           # angular resolution
H = 48          # spatial h/w per view
C = 64          # channels (ci = co = 64)
B = 4           # batch
ND = 9          # disparities -4..4
HH = 24         # h rows per core (half)
SUB = 8         # output h rows per slot
NSUB = HH // SUB
NSLOT = 41      # weight slots per row-half per plane
N_CORES = 8
NSPLIT = 2      # tile subsets (DMA overlap granularity)

BF16 = ml_dtypes.bfloat16


def _geometry():
    """Static tap/tile geometry shared by host packing and device program."""
    pairs = []            # (viewA, viewB-or-None, R)
    # the four v=4 u-pairs are seed-capable (no w-shift for any d, full-box
    # in-tile rows): keep them at known indices 0,1 and (built later) 18,19
    pairs.append(((0, 4), (8, 4), HH + 8 * 4))
    pairs.append(((1, 4), (7, 4), HH + 8 * 3))
    for v in range(A):
        for u in range(4):
            if v == 4 and u in (0, 1):
                continue
            pairs.append(((u, v), (8 - u, v), HH + 8 * (4 - u)))
    for k in range(4):
        pairs.append(((4, k), (4, k + 5), HH))
    pairs.append(((4, 4), None, HH))

    view_loc = {}
    offs = []
    off = 0
    for j, (va, vb, R) in enumerate(pairs):
        view_loc[va] = (j, 0)
        if vb is not None:
            view_loc[vb] = (j, 1)
        offs.append(off)
        off += R * H
    F = off

    # per-tap static info in tile order: (u, v, rh, s); s = weight column
    taps = []
    slot_ctr = {0: 0, 1: 0}
    for j, (va, vb, R) in enumerate(pairs):
        for half, view in ((0, va), (1, vb)):
            if view is None:
                continue
            u, v = view
            s = slot_ctr[half]
            slot_ctr[half] += 1
            taps.append((u, v, half, s, j))
    assert len(taps) == 81
    return pairs, view_loc, offs, F, taps


_PAIRS, _VIEW_LOC, _OFFS, _F, _TAPS = _geometry()
_TAP_BY_PAIR = {}
for _t in _TAPS:
    _TAP_BY_PAIR.setdefault(_t[4], []).append(_t)

# tile subsets, each starting with a seed pair (v=4 u-pair: indices 0,1,18,19)
_SEEDS = [0, 1, 18][:NSPLIT]
_REST = [j for j in range(len(_PAIRS)) if j not in _SEEDS]
_TILESETS = []
_chunk = (len(_REST) + NSPLIT - 1) // NSPLIT
for i in range(NSPLIT):
    _TILESETS.append([_SEEDS[i]] + _REST[i * _chunk:(i + 1) * _chunk])

# slots: (di, sub); planes: neg = d<=0 (unflipped W), pos = d>0 (flipped)
_NEG = [(di, s) for di in range(5) for s in range(NSUB)]      # 15
_POS = [(di, s) for di in range(5, ND) for s in range(NSUB)]  # 12
# phase triples: diag slots ride PE col group ch=rh, anti ride ch=1-rh.
# Each stream must be plane-pure (one LDW per tap per position).
_TRIPLES = [
    dict(diag=[_NEG[0], _NEG[2], _NEG[4]], anti=[_NEG[1], _NEG[3], _NEG[5]]),
    dict(diag=[_NEG[6], _NEG[8], _NEG[10]], anti=[_NEG[7], _NEG[9], _NEG[11]]),
    dict(diag=[_NEG[12], _NEG[13], _NEG[14]], anti=[_POS[0], _POS[1], _POS[2]]),
    dict(diag=[_POS[3], _POS[5], _POS[11]], anti=[_POS[4], _POS[6]]),
    dict(diag=[_POS[7], _POS[9]], anti=[_POS[8], _POS[10]]),
]
assert sorted(sum((t["diag"] + t["anti"] for t in _TRIPLES), [])) == \
    sorted(_NEG + _POS)

_NC_CACHE = {}


def _plane(di):
    return 0 if di <= 4 else 1


def _clip(di, sub, u, v):
    """Row/col clip for tap (u,v) at slot (di,sub): returns
    (row0, r_lo, r_hi, wlo, whi, sv) in-tile coords, or None if empty.
    Rows clip only where the shifted input row leaves the 48-row view
    (there x is zero-padded; works only because every out row was seeded)."""
    d = di - 4
    au = abs(4 - u)
    row0 = sub * SUB + d * (4 - u) + 4 * au
    sv = d * (4 - v)
    wlo = max(0, -sv)
    whi = min(H, H - sv)
    return row0, wlo, whi, sv


def _hclip(di, sub, u, h0):
    """Row clip is disabled: the SPMD program serves both h-halves, and the
    union of their valid row ranges always covers the full 8-row block, so
    out-of-image rows (zero-padded in SBUF) are simply multiplied through."""
    return 0, SUB


def _build_nc(repeat=1):
    import concourse.bacc as bacc
    import concourse.mybir as mybir
    import concourse.tile as tile

    nc = bacc.Bacc(None, target_bir_lowering=False)
    xwin_d = nc.dram_tensor("xwin", [128, _F], mybir.dt.bfloat16,
                            kind="ExternalInput")
    wp_d = nc.dram_tensor("wpack", [128, 2 * NSLOT * C], mybir.dt.bfloat16,
                          kind="ExternalInput")
    # h0 = this core's first output row (0 or 24): baked per-core via hsel
    # one-hot applied host-side?  No -- h0 affects h-clip which is static
    # per program.  Build with h0 as a compile-time constant is impossible
    # for SPMD (one program, 8 cores).  Use the WIDER clip (union of both
    # halves) so the program is core-independent: rows outside the core's
    # own window read zero-padded tile rows and contribute zeros.
    out_d = nc.dram_tensor("out", [C, ND * NSUB * SUB * H], mybir.dt.bfloat16,
                           kind="ExternalOutput")

    with tile.TileContext(nc) as tc:
        with tc.tile_pool(name="xw", bufs=1) as xpool, \
             tc.tile_pool(name="ac", bufs=1) as apool, \
             tc.tile_pool(name="ps", bufs=1, space="PSUM") as ppool, \
             tc.tile_pool(name="ob", bufs=4) as opool:

            wt = xpool.tile([128, 2 * NSLOT * C], mybir.dt.bfloat16, tag="wt")
            nc.sync.dma_start(out=wt[:], in_=wp_d[:, :])

            xtiles = [None] * len(_PAIRS)
            xviews = [None] * len(_PAIRS)
            for ts in _TILESETS:
                for j in ts:
                    va, vb, R = _PAIRS[j]
                    t = xpool.tile([128, R * H], mybir.dt.bfloat16,
                                   name=f"x{j}", tag=f"x{j}")
                    nc.sync.dma_start(
                        out=t[:], in_=xwin_d[:, _OFFS[j]:_OFFS[j] + R * H])
                    xtiles[j] = t
                    xviews[j] = t[:].rearrange("p (r c) -> p r c", r=R, c=H)

            accs = {}
            for di in range(ND):
                for s in range(NSUB):
                    accs[(di, s)] = apool.tile(
                        [64, SUB * H], mybir.dt.bfloat16,
                        name=f"a{di}_{s}", tag=f"a{di}_{s}")

            bank_ctr = [0]

            def emit_phase(tri, ts_idx, body_state):
                slots = list(tri["diag"]) + list(tri["anti"])
                nslot = len(slots)
                ptiles = {}
                for i, sl in enumerate(slots):
                    tag = f"b{(bank_ctr[0] + i) % 8}"
                    ptiles[sl] = ppool.tile([128, SUB * H], mybir.dt.float32,
                                            name=f"p{ts_idx}_{sl[0]}_{sl[1]}",
                                            tag=tag)
                bank_ctr[0] += nslot

                started = set()
                for j in _TILESETS[ts_idx]:
                    for (u, v, rh, s, _) in _TAP_BY_PAIR[j]:
                        for group, chsel in ((tri["diag"], 0), (tri["anti"], 1)):
                            ch = rh if chsel == 0 else 1 - rh
                            emitted_lhs = None
                            for sl in group:
                                di, sub = sl
                                pl = _plane(di)
                                row0, wlo, whi, sv = _clip(di, sub, u, v)
                                seed = (j == _TILESETS[ts_idx][0])
                                if seed:
                                    r_lo, r_hi = 0, SUB
                                    wl, wh = 0, H
                                else:
                                    r_lo, r_hi = _hclip(di, sub, u, 0)
                                    wl, wh = wlo, whi
                                lhsT = wt[rh * 64:(rh + 1) * 64,
                                          (pl * NSLOT + s) * C:
                                          (pl * NSLOT + s + 1) * C]
                                rhs = xviews[j][rh * 64:(rh + 1) * 64,
                                                row0 + r_lo:row0 + r_hi,
                                                wl + sv:wh + sv]
                                pt = ptiles[sl]
                                if r_lo == 0 and r_hi == SUB and \
                                        wl == 0 and wh == H:
                                    outap = pt[ch * 64:(ch + 1) * 64, :]
                                else:
                                    ptv = pt[:].rearrange(
                                        "p (r c) -> p r c", r=SUB, c=H)
                                    outap = ptv[ch * 64:(ch + 1) * 64,
                                                r_lo:r_hi, wl:wh]
                                key = (sl, ch)
                                nc.tensor.matmul(
                                    outap, lhsT, rhs,
                                    start=(key not in started),
                                    stop=False,
                                    tile_position=(rh * 64, ch * 64),
                                    skip_group_check=True,
                                )
                                started.add(key)
                                emitted_lhs = lhsT

                # flush / final evac
                for sl in slots:
                    di, sub = sl
                    pt = ptiles[sl]
                    acc = accs[sl]
                    if ts_idx == 0 and NSPLIT > 1:
                        nc.vector.tensor_copy(acc[:], pt[0:64, :])
                        nc.vector.tensor_add(acc[:], acc[:], pt[64:128, :])
                    elif ts_idx < NSPLIT - 1:
                        nc.vector.tensor_add(acc[:], acc[:], pt[0:64, :])
                        nc.vector.tensor_add(acc[:], acc[:], pt[64:128, :])
                    else:
                        ot = opool.tile([64, SUB * H], mybir.dt.bfloat16,
                                        tag="ot")
                        if NSPLIT > 1:
                            nc.vector.tensor_add(ot[:], acc[:], pt[0:64, :])
                        else:
                            nc.vector.tensor_copy(ot[:], pt[0:64, :])
                        nc.vector.tensor_add(ot[:], ot[:], pt[64:128, :])
                        seg = (di * NSUB + sub) * SUB * H
                        nc.sync.dma_start(out=out_d[:, seg:seg + SUB * H],
                                          in_=ot[:])

            def body():
                for ts_idx in range(NSPLIT):
                    for tri in _TRIPLES:
                        emit_phase(tri, ts_idx, None)

            if repeat == 1:
                body()
            else:
                with tc.For_i(0, repeat, 1):
                    body()

    _dedup_ldweights(nc)
    nc.finalize()
    return nc


def _dedup_ldweights(nc):
    """Remove InstLdweights that reload the stationary operand already
    resident at the same tile position (slot-group MMs share tap weights)."""
    removed = kept = 0
    for bb in nc.m.functions[0].blocks:
        last = {}
        to_remove = []
        for ins in bb.instructions:
            if not str(ins.engine).endswith("PE"):
                continue
            tn = type(ins).__name__
            if tn == "InstLdweights":
                si = ins.sync_info
                has_sync = si is not None and (si.on_wait or si.on_update)
                sig = (str(ins.ins[0]), str(getattr(ins, "tile_position", None)),
                       str(getattr(ins, "perf_mode", None)))
                pos = str(getattr(ins, "tile_position", None))
                if not has_sync and last.get(pos) == sig:
                    to_remove.append(ins)
                    removed += 1
                else:
                    last[pos] = sig
                    kept += 1
            elif tn == "InstMatmult":
                continue
            else:
                last.clear()
        for ins in to_remove:
            bb.instructions.remove(ins)
    if removed:
        import logging
        logging.getLogger(__name__).info(
            "dedup_ldweights: removed %d, kept %d", removed, kept)


def get_nc(repeat=1):
    key = ("nc", repeat, NSPLIT)
    if key not in _NC_CACHE:
        _NC_CACHE[key] = _build_nc(repeat)
    return _NC_CACHE[key]


def prepare_inputs(x, W):
    """Host-side packing: per-core xwin [128,F] bf16 + shared 2-plane wpack."""
    x = np.asarray(x, dtype=np.float32)
    W = np.asarray(W, dtype=np.float32)
    X5 = np.ascontiguousarray(
        x.reshape(B, C, H, A, H, A).transpose(0, 3, 5, 1, 2, 4)
    ).astype(BF16)

    xwins = []
    for core in range(N_CORES):
        b, hh = divmod(core, 2)
        h0 = hh * HH
        xw = np.zeros((128, _F), dtype=BF16)
        for j, (va, vb, R) in enumerate(_PAIRS):
            for half, view in ((0, va), (1, vb)):
                if view is None:
                    continue
                u, v = view
                lo = h0 - 4 * abs(4 - u)
                vs = max(0, lo)
                ve = min(H, lo + R)
                blk = X5[b, u, v, :, vs:ve, :]
                dst = xw[half * 64:(half + 1) * 64,
                         _OFFS[j]:_OFFS[j] + R * H].reshape(64, R, H)
                dst[:, vs - lo:ve - lo, :] = blk
        xwins.append(xw)

    wpack = np.zeros((128, 2 * NSLOT * C), dtype=BF16)
    Wb = W.astype(BF16)
    for (u, v, rh, s, _) in _TAPS:
        for pl in range(2):
            kh, kw = (u, v) if pl == 0 else (8 - u, 8 - v)
            wpack[rh * 64:(rh + 1) * 64,
                  (pl * NSLOT + s) * C:(pl * NSLOT + s + 1) * C] = \
                Wb[:, :, kh, kw].T
    return xwins, wpack


def assemble_output(results):
    """results: list of 8 dicts with 'out' [64, ND*NSUB*SUB*H] bf16."""
    full = np.empty((B, C, ND, H, H), dtype=np.float32)
    for core in range(N_CORES):
        b, hh = divmod(core, 2)
        oc = np.asarray(results[core]["out"]).astype(np.float32)
        oc = oc.reshape(C, ND, HH, H)
        full[b, :, :, hh * HH:(hh + 1) * HH, :] = oc
    return full


def kernel(x, W):
    from concourse.bass_utils import run_bass_kernel_spmd

    nc = get_nc()
    xwins, wpack = prepare_inputs(x, W)
    in_maps = [{"xwin": xwins[c], "wpack": wpack} for c in range(N_CORES)]
    res = run_bass_kernel_spmd(nc, in_maps, core_ids=list(range(N_CORES)))
    return assemble_output(res.results)
